# revision 1
# baseline (speedup 1.0000x reference)
"""AblationRouter (EMA scan + predictor MLP + linear router) on 8 TRN2 cores.

Strategy: data-parallel over the batch (B=16 -> 2 rows per core), weights
replicated and streamed from HBM per 512-token chunk.

Per-core device program (Bass/Tile):
  x [2*1024, 1024] f32 (time-major, zero-padded t=1023)
    --GpSimd cast--> f16 --PE transpose-mode--> xT [d, t] (PSUM, f16)
  EMA scan: DVE tensor_tensor_scan along time (fp32 state, per-channel
    beta, carry chained across chunks) -> h [d, t] f16
  inp f16 [128, 16 k-tiles, 512]  (8 x-tiles + 8 h-tiles)
  MLP1: hid = relu(p1_wT.T @ inp + b1)  (f16 matmuls, f32 PSUM accum)
  MLP2: x_hat = p2_wT.T @ hid + b2      (f32 out + f16 copy)
  router: logits = (W_wT.T @ x_hat + W_b) * pi  (folded into MLP2 loop)
Outputs per core: xhat [1024, 2048] f32, logits [64, 2048] f32
(feature-major; host transposes/reassembles).
"""
from contextlib import ExitStack

import numpy as np

import concourse.bacc as bacc
import concourse.tile as tile
from concourse import mybir
from concourse.bass_utils import run_bass_kernel_spmd

F32 = mybir.dt.float32
F16 = mybir.dt.float16
AF = mybir.ActivationFunctionType
ALU = mybir.AluOpType

N_CORES = 8
T_PAD = 1024
CHUNKS = (512, 512)


def build(B_loc=2, T_pad=T_PAD, D=1024, H=4096, E=64, n_cores=N_CORES,
          chunk_sizes=CHUNKS, use_dma_tp=False, w1_hoist=6):
    KD = D // 128
    KH = H // 128
    BT = B_loc * T_pad
    assert sum(chunk_sizes) == T_pad

    nc = bacc.Bacc("TRN2", target_bir_lowering=False, debug=False,
                   num_devices=n_cores)

    x_d = nc.dram_tensor("x", [BT, D], F32, kind="ExternalInput").ap()
    p1_d = nc.dram_tensor("p1wt", [2 * D, H], F16, kind="ExternalInput").ap()
    p2_d = nc.dram_tensor("p2wt", [H, D], F16, kind="ExternalInput").ap()
    wr_d = nc.dram_tensor("wrt", [D, E], F16, kind="ExternalInput").ap()
    beta_d = nc.dram_tensor("beta", [128, KD], F32, kind="ExternalInput").ap()
    b1_d = nc.dram_tensor("b1", [128, KH], F32, kind="ExternalInput").ap()
    b2_d = nc.dram_tensor("b2", [128, KD], F32, kind="ExternalInput").ap()
    wbpi_d = nc.dram_tensor("wbpi", [E, 1], F32, kind="ExternalInput").ap()
    pi_d = nc.dram_tensor("pi", [E, 1], F32, kind="ExternalInput").ap()
    id_d = nc.dram_tensor("ident", [128, 128], F16, kind="ExternalInput").ap()

    xhat_d = nc.dram_tensor("xhat", [D, BT], F32, kind="ExternalOutput").ap()
    lg_d = nc.dram_tensor("logits", [E, BT], F32, kind="ExternalOutput").ap()

    with tile.TileContext(nc) as tc, ExitStack() as ctx:
        consts = ctx.enter_context(tc.tile_pool(name="consts", bufs=1))
        xraw_p = ctx.enter_context(tc.tile_pool(name="xraw", bufs=2))
        x16_p = ctx.enter_context(tc.tile_pool(name="x16", bufs=2))
        inp_p = ctx.enter_context(tc.tile_pool(name="inp", bufs=2))
        hid_p = ctx.enter_context(tc.tile_pool(name="hid", bufs=1))
        xh32_p = ctx.enter_context(tc.tile_pool(name="xh32", bufs=1))
        xh16_p = ctx.enter_context(tc.tile_pool(name="xh16", bufs=1))
        lgs_p = ctx.enter_context(tc.tile_pool(name="lgs", bufs=2))
        w1_p = ctx.enter_context(tc.tile_pool(name="w1", bufs=6))
        w2_p = ctx.enter_context(tc.tile_pool(name="w2", bufs=3))
        tp_ps = ctx.enter_context(tc.tile_pool(name="tps", bufs=2, space="PSUM"))
        h_ps = ctx.enter_context(tc.tile_pool(name="hps", bufs=2, space="PSUM"))
        x_ps = ctx.enter_context(tc.tile_pool(name="xps", bufs=2, space="PSUM"))
        l_ps = ctx.enter_context(tc.tile_pool(name="lps", bufs=2, space="PSUM"))

        ident = consts.tile([128, 128], F16, tag="ident")
        nc.sync.dma_start(ident[:], id_d)
        beta_t = consts.tile([128, KD], F32, tag="beta")
        nc.sync.dma_start(beta_t[:], beta_d)
        b1_t = consts.tile([128, KH], F32, tag="b1")
        nc.sync.dma_start(b1_t[:], b1_d)
        b2_t = consts.tile([128, KD], F32, tag="b2")
        nc.sync.dma_start(b2_t[:], b2_d)
        wbpi_t = consts.tile([E, 1], F32, tag="wbpi")
        nc.sync.dma_start(wbpi_t[:], wbpi_d)
        pi_t = consts.tile([E, 1], F32, tag="pi")
        nc.sync.dma_start(pi_t[:], pi_d)
        wr_t = consts.tile([128, KD, E], F16, tag="wr")
        nc.sync.dma_start(wr_t[:], wr_d.rearrange("(kd p) e -> p kd e", p=128))
        carries = [consts.tile([128, KD], F16, tag=f"carry{r}")
                   for r in range(B_loc)]

        chunk_list = [(r, ci) for ci in range(len(chunk_sizes))
                      for r in range(B_loc)]

        for (row, ci) in chunk_list:
            NCH = chunk_sizes[ci]
            TB = NCH // 128
            toff = sum(chunk_sizes[:ci])
            col0 = row * T_pad + toff

            xraw = xraw_p.tile([128, TB, D], F32, tag="xraw")
            x16 = x16_p.tile([128, TB, D], F16, tag="x16")
            for tb in range(TB):
                src = x_d[col0 + tb * 128: col0 + (tb + 1) * 128, :]
                nc.sync.dma_start(xraw[:, tb, :], src)
                nc.gpsimd.tensor_copy(x16[:, tb, :], xraw[:, tb, :])

            inp = inp_p.tile([128, 2 * KD, NCH], F16, tag="inp")

            # hoisted w1 prefetch for the first m-tiles of this chunk
            w1_tiles = {}
            for m in range(min(w1_hoist, KH)):
                w1 = w1_p.tile([128, 2 * KD, 128], F16, tag="w1")
                nc.sync.dma_start(
                    w1[:],
                    p1_d[:, m * 128:(m + 1) * 128].rearrange(
                        "(kt p) m -> p kt m", p=128))
                w1_tiles[m] = w1

            for dt in range(KD):
                if use_dma_tp:
                    for tb in range(TB):
                        nc.sync.dma_start_transpose(
                            inp[:, dt, tb * 128:(tb + 1) * 128],
                            x16[:, tb, dt * 128:(dt + 1) * 128])
                    scan_src = inp[:, dt, :]
                else:
                    tp = tp_ps.tile([128, NCH], F16, tag="tp")
                    for tb in range(TB):
                        nc.tensor.transpose(
                            tp[:, tb * 128:(tb + 1) * 128],
                            x16[:, tb, dt * 128:(dt + 1) * 128],
                            ident[:])
                    nc.scalar.activation(inp[:, dt, :], tp[:], AF.Copy)
                    scan_src = tp[:]
                init = 0.0 if ci == 0 else carries[row][:, dt:dt + 1]
                nc.vector.tensor_tensor_scan(
                    inp[:, KD + dt, :],
                    beta_t[:, dt:dt + 1].broadcast_to([128, NCH]), scan_src,
                    init, ALU.mult, ALU.add)
                if ci + 1 < len(chunk_sizes):
                    nc.vector.tensor_copy(carries[row][:, dt:dt + 1],
                                          inp[:, KD + dt, NCH - 1:NCH])

            hid = hid_p.tile([128, KH, NCH], F16, tag="hid")
            for m in range(KH):
                if m in w1_tiles:
                    w1 = w1_tiles.pop(m)
                else:
                    w1 = w1_p.tile([128, 2 * KD, 128], F16, tag="w1")
                    nc.sync.dma_start(
                        w1[:],
                        p1_d[:, m * 128:(m + 1) * 128].rearrange(
                            "(kt p) m -> p kt m", p=128))
                hp = h_ps.tile([128, NCH], F32, tag="hp")
                for kt in range(2 * KD):
                    nc.tensor.matmul(hp[:], w1[:, kt, :], inp[:, kt, :],
                                     start=(kt == 0), stop=(kt == 2 * KD - 1))
                nc.scalar.activation(hid[:, m, :], hp[:], AF.Relu,
                                     bias=b1_t[:, m:m + 1])

            xh32 = xh32_p.tile([128, KD, NCH], F32, tag="xh32")
            xh16 = xh16_p.tile([128, KD, NCH], F16, tag="xh16")
            lp = l_ps.tile([E, NCH], F32, tag="lp")
            for m2 in range(KD):
                w2 = w2_p.tile([128, KH, 128], F16, tag="w2")
                nc.sync.dma_start(
                    w2[:],
                    p2_d[:, m2 * 128:(m2 + 1) * 128].rearrange(
                        "(kt p) m -> p kt m", p=128))
                xp = x_ps.tile([128, NCH], F32, tag="xp")
                for kt in range(KH):
                    nc.tensor.matmul(xp[:], w2[:, kt, :], hid[:, kt, :],
                                     start=(kt == 0), stop=(kt == KH - 1))
                nc.scalar.activation(xh32[:, m2, :], xp[:], AF.Identity,
                                     bias=b2_t[:, m2:m2 + 1])
                nc.vector.tensor_scalar_add(xh16[:, m2, :], xp[:],
                                            b2_t[:, m2:m2 + 1])
                nc.tensor.matmul(lp[:], wr_t[:, m2, :], xh16[:, m2, :],
                                 start=(m2 == 0), stop=(m2 == KD - 1))
                nc.sync.dma_start(
                    xhat_d[m2 * 128:(m2 + 1) * 128, col0:col0 + NCH],
                    xh32[:, m2, :])

            lgs = lgs_p.tile([E, NCH], F32, tag="lgs")
            nc.scalar.activation(lgs[:], lp[:], AF.Identity,
                                 bias=wbpi_t[:], scale=pi_t[:])
            nc.sync.dma_start(lg_d[:, col0:col0 + NCH], lgs[:])

    nc.compile()
    return nc


def prep_core_inputs(seq_core, shared, T_pad=T_PAD):
    """Per-core inputs: x (padded, time-major) + shared preprocessed weights."""
    B_loc, T, D = seq_core.shape
    x = np.zeros((B_loc * T_pad, D), np.float32)
    for r in range(B_loc):
        x[r * T_pad:r * T_pad + T - 1] = seq_core[r, :T - 1]
    return {"x": x, **shared}


def prep_shared(pi, beta_raw, p1_w, p1_b, p2_w, p2_b, W_w, W_b):
    D = beta_raw.shape[0]
    H = p1_w.shape[0]
    E = W_w.shape[0]
    KD, KH = D // 128, H // 128
    beta = (1.0 / (1.0 + np.exp(-beta_raw.astype(np.float64)))).astype(np.float32)
    return {
        "p1wt": np.ascontiguousarray(p1_w.T).astype(np.float16),
        "p2wt": np.ascontiguousarray(p2_w.T).astype(np.float16),
        "wrt": np.ascontiguousarray(W_w.T).astype(np.float16),
        "beta": np.ascontiguousarray(beta.reshape(KD, 128).T),
        "b1": np.ascontiguousarray(
            np.asarray(p1_b, np.float32).reshape(KH, 128).T),
        "b2": np.ascontiguousarray(
            np.asarray(p2_b, np.float32).reshape(KD, 128).T),
        "wbpi": (np.asarray(W_b, np.float32)
                 * np.asarray(pi, np.float32)).reshape(E, 1),
        "pi": np.asarray(pi, np.float32).reshape(E, 1),
        "ident": np.eye(128, dtype=np.float16),
    }


def postprocess(results, B, T, D, E, T_pad=T_PAD):
    n_cores = len(results)
    B_loc = B // n_cores
    logits = np.empty((T - 1, B, E), np.float32)
    x_hat = np.empty((T - 1, B, D), np.float32)
    for c, res in enumerate(results):
        xh = res["xhat"].reshape(D, B_loc, T_pad)
        lg = res["logits"].reshape(E, B_loc, T_pad)
        for r in range(B_loc):
            x_hat[:, c * B_loc + r, :] = xh[:, r, :T - 1].T
            logits[:, c * B_loc + r, :] = lg[:, r, :T - 1].T
    return logits, x_hat


_NC_CACHE = {}


def _get_nc(B_loc, T_pad, D, H, E, n_cores):
    key = (B_loc, T_pad, D, H, E, n_cores)
    if key not in _NC_CACHE:
        _NC_CACHE[key] = build(B_loc=B_loc, T_pad=T_pad, D=D, H=H, E=E,
                               n_cores=n_cores)
    return _NC_CACHE[key]


def kernel(seq, pi, beta_raw, p1_w, p1_b, p2_w, p2_b, W_w, W_b):
    seq = np.asarray(seq, np.float32)
    B, T, D = seq.shape
    H = np.asarray(p1_w).shape[0]
    E = np.asarray(W_w).shape[0]
    n_cores = N_CORES
    assert B % n_cores == 0
    B_loc = B // n_cores

    nc = _get_nc(B_loc, T_PAD, D, H, E, n_cores)
    shared = prep_shared(np.asarray(pi), np.asarray(beta_raw),
                         np.asarray(p1_w), np.asarray(p1_b),
                         np.asarray(p2_w), np.asarray(p2_b),
                         np.asarray(W_w), np.asarray(W_b))
    in_maps = [prep_core_inputs(seq[c * B_loc:(c + 1) * B_loc], shared)
               for c in range(n_cores)]

    res = run_bass_kernel_spmd(nc, in_maps,
                               core_ids=list(range(n_cores))).results
    logits, x_hat = postprocess(res, B, T, D, E)
    return logits, x_hat


# revision 2
# speedup vs baseline: 1.1951x; 1.1951x over previous
"""AblationRouter (EMA scan + predictor MLP + linear router) on 8 TRN2 cores.

Sharding: data-parallel over batch (B=16 -> 2 rows per core); router/MLP
weights replicated, streamed from HBM once per 512-token time-half with the
two batch rows sharing each loaded weight tile.

Per-core device program (Bass/Tile):
  x [2*1024, 1024] f16 (time-major, zero-padded at t=1023)
    --PE transpose-mode--> xT [d, t] (PSUM, f16)
  EMA scan: DVE tensor_tensor_scan along time (fp32 state, per-channel
    beta, carry chained across chunks) -> h [d, t] f16
  inp f16 [128, 16 k-tiles, 512] = [x-tiles | h-tiles]
  MLP1: hid = relu(p1_wT.T @ inp + b1)   (f16 matmuls, f32 PSUM accum)
  MLP2: x_hat = p2_wT.T @ hid + b2       (f32 out + f16 copy)
  router (folded into MLP2 loop): logits = (W_wT.T @ x_hat + W_b) * pi
Outputs per core: xhat [1024, 2048] f32, logits [64, 2048] f32
(feature-major; host reassembles to (T-1, B, ...)).
"""
from contextlib import ExitStack

import numpy as np

import concourse.bacc as bacc
import concourse.tile as tile
from concourse import mybir
from concourse.bass_utils import run_bass_kernel_spmd

F32 = mybir.dt.float32
F16 = mybir.dt.float16
AF = mybir.ActivationFunctionType
ALU = mybir.AluOpType

N_CORES = 8
T_PAD = 1024
CHUNKS = (512, 512)


def build(B_loc=2, T_pad=1024, D=1024, H=4096, E=64, n_cores=8,
          chunk_sizes=(512, 512), w1_hoist=6):
    KD = D // 128
    KH = H // 128
    BT = B_loc * T_pad
    assert sum(chunk_sizes) == T_pad
    assert B_loc == 2

    nc = bacc.Bacc("TRN2", target_bir_lowering=False, debug=False,
                   num_devices=n_cores)

    x_d = nc.dram_tensor("x", [BT, D], F32, kind="ExternalInput").ap()
    p1_d = nc.dram_tensor("p1wt", [2 * D, H], F16, kind="ExternalInput").ap()
    p2_d = nc.dram_tensor("p2wt", [H, D], F16, kind="ExternalInput").ap()
    wr_d = nc.dram_tensor("wrt", [D, E], F16, kind="ExternalInput").ap()
    beta_d = nc.dram_tensor("beta", [128, KD], F32, kind="ExternalInput").ap()
    b1_d = nc.dram_tensor("b1", [128, KH], F32, kind="ExternalInput").ap()
    b2_d = nc.dram_tensor("b2", [128, KD], F32, kind="ExternalInput").ap()
    wbpi_d = nc.dram_tensor("wbpi", [E, 1], F32, kind="ExternalInput").ap()
    pi_d = nc.dram_tensor("pi", [E, 1], F32, kind="ExternalInput").ap()
    id_d = nc.dram_tensor("ident", [128, 128], F16, kind="ExternalInput").ap()

    xhat_d = nc.dram_tensor("xhat", [D, BT], F32, kind="ExternalOutput").ap()
    lg_d = nc.dram_tensor("logits", [E, BT], F32, kind="ExternalOutput").ap()

    with tile.TileContext(nc) as tc, ExitStack() as ctx:
        consts = ctx.enter_context(tc.tile_pool(name="consts", bufs=1))
        xraw_p = ctx.enter_context(tc.tile_pool(name="xraw", bufs=3))
        x16_p = ctx.enter_context(tc.tile_pool(name="x16", bufs=2))
        inp_p = ctx.enter_context(tc.tile_pool(name="inp", bufs=2))
        hid_p = ctx.enter_context(tc.tile_pool(name="hid", bufs=2))
        xh32_p = ctx.enter_context(tc.tile_pool(name="xh32", bufs=3))
        xh16_p = ctx.enter_context(tc.tile_pool(name="xh16", bufs=3))
        lgs_p = ctx.enter_context(tc.tile_pool(name="lgs", bufs=2))
        w1_p = ctx.enter_context(tc.tile_pool(name="w1", bufs=6))
        w2_p = ctx.enter_context(tc.tile_pool(name="w2", bufs=3))
        tp_ps = ctx.enter_context(tc.tile_pool(name="tps", bufs=2, space="PSUM"))
        h_ps = ctx.enter_context(tc.tile_pool(name="hps", bufs=2, space="PSUM"))
        x_ps = ctx.enter_context(tc.tile_pool(name="xps", bufs=2, space="PSUM"))
        l_ps = ctx.enter_context(tc.tile_pool(name="lps", bufs=2, space="PSUM"))

        ident = consts.tile([128, 128], F16, tag="ident")
        nc.sync.dma_start(ident[:], id_d)
        beta_t = consts.tile([128, KD], F32, tag="beta")
        nc.sync.dma_start(beta_t[:], beta_d)
        b1_t = consts.tile([128, KH], F32, tag="b1")
        nc.sync.dma_start(b1_t[:], b1_d)
        b2_t = consts.tile([128, KD], F32, tag="b2")
        nc.sync.dma_start(b2_t[:], b2_d)
        wbpi_t = consts.tile([E, 1], F32, tag="wbpi")
        nc.sync.dma_start(wbpi_t[:], wbpi_d)
        pi_t = consts.tile([E, 1], F32, tag="pi")
        nc.sync.dma_start(pi_t[:], pi_d)
        wr_t = consts.tile([128, KD, E], F16, tag="wr")
        nc.sync.dma_start(wr_t[:], wr_d.rearrange("(kd p) e -> p kd e", p=128))
        carries = [consts.tile([128, KD], F16, tag=f"carry{r}")
                   for r in range(B_loc)]

        for ci in range(len(chunk_sizes)):
            NCH = chunk_sizes[ci]
            TB = NCH // 128
            toff = sum(chunk_sizes[:ci])
            col0s = [r * T_pad + toff for r in range(B_loc)]

            # ---- inp build for both rows ----
            x16s = []
            inps = []
            for r in range(B_loc):
                x16 = x16_p.tile([128, TB, D], F16, tag="x16")
                for tb in range(TB):
                    xraw = xraw_p.tile([128, D], F32, tag="xraw")
                    nc.sync.dma_start(
                        xraw[:],
                        x_d[col0s[r] + tb * 128: col0s[r] + (tb + 1) * 128, :])
                    nc.gpsimd.tensor_copy(x16[:, tb, :], xraw[:])
                x16s.append(x16)
                inps.append(inp_p.tile([128, 2 * KD, NCH], F16, tag="inp"))

            # hoisted w1 prefetch
            w1_tiles = {}
            for m in range(min(w1_hoist, KH)):
                w1 = w1_p.tile([128, 2 * KD, 128], F16, tag="w1")
                nc.sync.dma_start(
                    w1[:],
                    p1_d[:, m * 128:(m + 1) * 128].rearrange(
                        "(kt p) m -> p kt m", p=128))
                w1_tiles[m] = w1

            for r in range(B_loc):
                inp, x16 = inps[r], x16s[r]
                for dt in range(KD):
                    tp = tp_ps.tile([128, NCH], F16, tag="tp")
                    for tb in range(TB):
                        nc.tensor.transpose(
                            tp[:, tb * 128:(tb + 1) * 128],
                            x16[:, tb, dt * 128:(dt + 1) * 128],
                            ident[:])
                    nc.scalar.activation(inp[:, dt, :], tp[:], AF.Copy)
                    init = 0.0 if ci == 0 else carries[r][:, dt:dt + 1]
                    nc.vector.tensor_tensor_scan(
                        inp[:, KD + dt, :],
                        beta_t[:, dt:dt + 1].broadcast_to([128, NCH]), tp[:],
                        init, ALU.mult, ALU.add)
                    if ci + 1 < len(chunk_sizes):
                        nc.vector.tensor_copy(carries[r][:, dt:dt + 1],
                                              inp[:, KD + dt, NCH - 1:NCH])

            # ---- MLP1 (shared weights) ----
            hids = [hid_p.tile([128, KH, NCH], F16, tag="hid")
                    for _ in range(B_loc)]
            for m in range(KH):
                if m in w1_tiles:
                    w1 = w1_tiles.pop(m)
                else:
                    w1 = w1_p.tile([128, 2 * KD, 128], F16, tag="w1")
                    nc.sync.dma_start(
                        w1[:],
                        p1_d[:, m * 128:(m + 1) * 128].rearrange(
                            "(kt p) m -> p kt m", p=128))
                for r in range(B_loc):
                    hp = h_ps.tile([128, NCH], F32, tag="hp")
                    for kt in range(2 * KD):
                        nc.tensor.matmul(hp[:], w1[:, kt, :], inps[r][:, kt, :],
                                         start=(kt == 0),
                                         stop=(kt == 2 * KD - 1))
                    nc.scalar.activation(hids[r][:, m, :], hp[:], AF.Relu,
                                         bias=b1_t[:, m:m + 1])

            # ---- MLP2 + router (shared weights) ----
            lps = [l_ps.tile([E, NCH], F32, tag="lp") for _ in range(B_loc)]
            for m2 in range(KD):
                w2 = w2_p.tile([128, KH, 128], F16, tag="w2")
                nc.sync.dma_start(
                    w2[:],
                    p2_d[:, m2 * 128:(m2 + 1) * 128].rearrange(
                        "(kt p) m -> p kt m", p=128))
                for r in range(B_loc):
                    xp = x_ps.tile([128, NCH], F32, tag="xp")
                    for kt in range(KH):
                        nc.tensor.matmul(xp[:], w2[:, kt, :], hids[r][:, kt, :],
                                         start=(kt == 0), stop=(kt == KH - 1))
                    xh32 = xh32_p.tile([128, NCH], F32, tag="xh32")
                    xh16 = xh16_p.tile([128, NCH], F16, tag="xh16")
                    nc.scalar.activation(xh32[:], xp[:], AF.Identity,
                                         bias=b2_t[:, m2:m2 + 1])
                    nc.vector.tensor_scalar_add(xh16[:], xp[:],
                                                b2_t[:, m2:m2 + 1])
                    nc.tensor.matmul(lps[r][:], wr_t[:, m2, :], xh16[:],
                                     start=(m2 == 0), stop=(m2 == KD - 1))
                    nc.sync.dma_start(
                        xhat_d[m2 * 128:(m2 + 1) * 128,
                               col0s[r]:col0s[r] + NCH],
                        xh32[:])

            for r in range(B_loc):
                lgs = lgs_p.tile([E, NCH], F32, tag="lgs")
                nc.scalar.activation(lgs[:], lps[r][:], AF.Identity,
                                     bias=wbpi_t[:], scale=pi_t[:])
                nc.sync.dma_start(lg_d[:, col0s[r]:col0s[r] + NCH], lgs[:])

    nc.compile()
    return nc


def prep_shared(pi, beta_raw, p1_w, p1_b, p2_w, p2_b, W_w, W_b):
    D = beta_raw.shape[0]
    H = p1_w.shape[0]
    E = W_w.shape[0]
    KD, KH = D // 128, H // 128
    beta = (1.0 / (1.0 + np.exp(-beta_raw.astype(np.float64)))).astype(np.float32)
    return {
        "p1wt": np.ascontiguousarray(p1_w.T).astype(np.float16),
        "p2wt": np.ascontiguousarray(p2_w.T).astype(np.float16),
        "wrt": np.ascontiguousarray(W_w.T).astype(np.float16),
        "beta": np.ascontiguousarray(beta.reshape(KD, 128).T),
        "b1": np.ascontiguousarray(
            np.asarray(p1_b, np.float32).reshape(KH, 128).T),
        "b2": np.ascontiguousarray(
            np.asarray(p2_b, np.float32).reshape(KD, 128).T),
        "wbpi": (np.asarray(W_b, np.float32)
                 * np.asarray(pi, np.float32)).reshape(E, 1),
        "pi": np.asarray(pi, np.float32).reshape(E, 1),
        "ident": np.eye(128, dtype=np.float16),
    }


def prep_core_inputs(seq_core, shared, T_pad=T_PAD):
    B_loc, T, D = seq_core.shape
    x = np.zeros((B_loc * T_pad, D), np.float16)
    for r in range(B_loc):
        x[r * T_pad:r * T_pad + T - 1] = seq_core[r, :T - 1].astype(np.float16)
    return {"x": x, **shared}


def postprocess(results, B, T, D, E, T_pad=T_PAD):
    n_cores = len(results)
    B_loc = B // n_cores
    logits = np.empty((T - 1, B, E), np.float32)
    x_hat = np.empty((T - 1, B, D), np.float32)
    for c, res in enumerate(results):
        xh = res["xhat"].reshape(D, B_loc, T_pad)
        lg = res["logits"].reshape(E, B_loc, T_pad)
        for r in range(B_loc):
            x_hat[:, c * B_loc + r, :] = xh[:, r, :T - 1].T
            logits[:, c * B_loc + r, :] = lg[:, r, :T - 1].T
    return logits, x_hat


_NC_CACHE = {}


def _get_nc(B_loc, T_pad, D, H, E, n_cores):
    key = (B_loc, T_pad, D, H, E, n_cores)
    if key not in _NC_CACHE:
        _NC_CACHE[key] = build(B_loc=B_loc, T_pad=T_pad, D=D, H=H, E=E,
                               n_cores=n_cores)
    return _NC_CACHE[key]


def kernel(seq, pi, beta_raw, p1_w, p1_b, p2_w, p2_b, W_w, W_b):
    seq = np.asarray(seq, np.float32)
    B, T, D = seq.shape
    H = np.asarray(p1_w).shape[0]
    E = np.asarray(W_w).shape[0]
    n_cores = N_CORES
    assert B % n_cores == 0 and B // n_cores == 2
    B_loc = B // n_cores

    nc = _get_nc(B_loc, T_PAD, D, H, E, n_cores)
    shared = prep_shared(np.asarray(pi), np.asarray(beta_raw),
                         np.asarray(p1_w), np.asarray(p1_b),
                         np.asarray(p2_w), np.asarray(p2_b),
                         np.asarray(W_w), np.asarray(W_b))
    in_maps = [prep_core_inputs(seq[c * B_loc:(c + 1) * B_loc], shared)
               for c in range(n_cores)]

    res = run_bass_kernel_spmd(nc, in_maps,
                               core_ids=list(range(n_cores))).results
    logits, x_hat = postprocess(res, B, T, D, E)
    return logits, x_hat


# revision 3
# speedup vs baseline: 1.2103x; 1.0127x over previous
"""AblationRouter (EMA scan + predictor MLP + linear router) on 8 TRN2 cores.

Sharding: data-parallel over batch (B=16 -> 2 rows per core); router/MLP
weights replicated, streamed from HBM once per 512-token time-half with the
two batch rows sharing each loaded weight tile.

Per-core device program (Bass/Tile):
  x [2*1024, 1024] f16 (time-major, zero-padded at t=1023)
    --PE transpose-mode--> xT [d, t] (PSUM, f16)
  EMA scan: DVE tensor_tensor_scan along time (fp32 state, per-channel
    beta, carry chained across chunks) -> h [d, t] f16
  inp f16 [128, 16 k-tiles, 512] = [x-tiles | h-tiles]
  MLP1: hid = relu(p1_wT.T @ inp + b1)   (f16 matmuls, f32 PSUM accum)
  MLP2: x_hat = p2_wT.T @ hid + b2       (f32 out + f16 copy)
  router (folded into MLP2 loop): logits = (W_wT.T @ x_hat + W_b) * pi
Outputs per core: xhat [1024, 2048] f32, logits [64, 2048] f32
(feature-major; host reassembles to (T-1, B, ...)).
"""
from contextlib import ExitStack

import numpy as np

import concourse.bacc as bacc
import concourse.tile as tile
from concourse import mybir
from concourse.bass_utils import run_bass_kernel_spmd

F32 = mybir.dt.float32
F16 = mybir.dt.float16
AF = mybir.ActivationFunctionType
ALU = mybir.AluOpType

N_CORES = 8
T_PAD = 1024
CHUNKS = (512, 512)


def build(B_loc=2, T_pad=1024, D=1024, H=4096, E=64, n_cores=8,
          chunk_sizes=(512, 512), w1_hoist=6):
    KD = D // 128
    KH = H // 128
    BT = B_loc * T_pad
    assert sum(chunk_sizes) == T_pad
    assert B_loc == 2

    nc = bacc.Bacc("TRN2", target_bir_lowering=False, debug=False,
                   num_devices=n_cores)

    x_d = nc.dram_tensor("x", [BT, D], F32, kind="ExternalInput").ap()
    p1_d = nc.dram_tensor("p1wt", [2 * D, H], F16, kind="ExternalInput").ap()
    p2_d = nc.dram_tensor("p2wt", [H, D], F16, kind="ExternalInput").ap()
    wr_d = nc.dram_tensor("wrt", [D, E], F16, kind="ExternalInput").ap()
    beta_d = nc.dram_tensor("beta", [128, KD], F32, kind="ExternalInput").ap()
    b1_d = nc.dram_tensor("b1", [128, KH], F32, kind="ExternalInput").ap()
    b2_d = nc.dram_tensor("b2", [128, KD], F32, kind="ExternalInput").ap()
    wbpi_d = nc.dram_tensor("wbpi", [E, 1], F32, kind="ExternalInput").ap()
    pi_d = nc.dram_tensor("pi", [E, 1], F32, kind="ExternalInput").ap()
    id_d = nc.dram_tensor("ident", [128, 128], F16, kind="ExternalInput").ap()

    xhat_d = nc.dram_tensor("xhat", [D, BT], F32, kind="ExternalOutput").ap()
    lg_d = nc.dram_tensor("logits", [E, BT], F32, kind="ExternalOutput").ap()

    with tile.TileContext(nc) as tc, ExitStack() as ctx:
        consts = ctx.enter_context(tc.tile_pool(name="consts", bufs=1))
        xraw_p = ctx.enter_context(tc.tile_pool(name="xraw", bufs=3))
        x16_p = ctx.enter_context(tc.tile_pool(name="x16", bufs=2))
        inp_p = ctx.enter_context(tc.tile_pool(name="inp", bufs=2))
        hid_p = ctx.enter_context(tc.tile_pool(name="hid", bufs=2))
        xh32_p = ctx.enter_context(tc.tile_pool(name="xh32", bufs=3))
        xh16_p = ctx.enter_context(tc.tile_pool(name="xh16", bufs=3))
        lgs_p = ctx.enter_context(tc.tile_pool(name="lgs", bufs=2))
        w1_p = ctx.enter_context(tc.tile_pool(name="w1", bufs=6))
        w2_p = ctx.enter_context(tc.tile_pool(name="w2", bufs=3))
        tp_ps = ctx.enter_context(tc.tile_pool(name="tps", bufs=2, space="PSUM"))
        h_ps = ctx.enter_context(tc.tile_pool(name="hps", bufs=2, space="PSUM"))
        x_ps = ctx.enter_context(tc.tile_pool(name="xps", bufs=2, space="PSUM"))
        l_ps = ctx.enter_context(tc.tile_pool(name="lps", bufs=2, space="PSUM"))

        ident = consts.tile([128, 128], F16, tag="ident")
        nc.sync.dma_start(ident[:], id_d)
        beta_t = consts.tile([128, KD], F32, tag="beta")
        nc.sync.dma_start(beta_t[:], beta_d)
        b1_t = consts.tile([128, KH], F32, tag="b1")
        nc.sync.dma_start(b1_t[:], b1_d)
        b2_t = consts.tile([128, KD], F32, tag="b2")
        nc.sync.dma_start(b2_t[:], b2_d)
        wbpi_t = consts.tile([E, 1], F32, tag="wbpi")
        nc.sync.dma_start(wbpi_t[:], wbpi_d)
        pi_t = consts.tile([E, 1], F32, tag="pi")
        nc.sync.dma_start(pi_t[:], pi_d)
        wr_t = consts.tile([128, KD, E], F16, tag="wr")
        nc.sync.dma_start(wr_t[:], wr_d.rearrange("(kd p) e -> p kd e", p=128))
        carries = [consts.tile([128, KD], F16, tag=f"carry{r}")
                   for r in range(B_loc)]

        for ci in range(len(chunk_sizes)):
            NCH = chunk_sizes[ci]
            TB = NCH // 128
            toff = sum(chunk_sizes[:ci])
            col0s = [r * T_pad + toff for r in range(B_loc)]

            # ---- inp build for both rows ----
            x16s = []
            inps = []
            for r in range(B_loc):
                x16 = x16_p.tile([128, TB, D], F16, tag="x16")
                for tb in range(TB):
                    xraw = xraw_p.tile([128, D], F32, tag="xraw")
                    nc.sync.dma_start(
                        xraw[:],
                        x_d[col0s[r] + tb * 128: col0s[r] + (tb + 1) * 128, :])
                    nc.gpsimd.tensor_copy(x16[:, tb, :], xraw[:])
                x16s.append(x16)
                inps.append(inp_p.tile([128, 2 * KD, NCH], F16, tag="inp"))

            # hoisted w1 prefetch
            w1_tiles = {}
            for m in range(min(w1_hoist, KH)):
                w1 = w1_p.tile([128, 2 * KD, 128], F16, tag="w1")
                nc.sync.dma_start(
                    w1[:],
                    p1_d[:, m * 128:(m + 1) * 128].rearrange(
                        "(kt p) m -> p kt m", p=128))
                w1_tiles[m] = w1

            for r in range(B_loc):
                inp, x16 = inps[r], x16s[r]
                for dt in range(KD):
                    tp = tp_ps.tile([128, NCH], F16, tag="tp")
                    for tb in range(TB):
                        nc.tensor.transpose(
                            tp[:, tb * 128:(tb + 1) * 128],
                            x16[:, tb, dt * 128:(dt + 1) * 128],
                            ident[:])
                    nc.scalar.activation(inp[:, dt, :], tp[:], AF.Copy)
                    init = 0.0 if ci == 0 else carries[r][:, dt:dt + 1]
                    nc.vector.tensor_tensor_scan(
                        inp[:, KD + dt, :],
                        beta_t[:, dt:dt + 1].broadcast_to([128, NCH]), tp[:],
                        init, ALU.mult, ALU.add)
                    if ci + 1 < len(chunk_sizes):
                        nc.vector.tensor_copy(carries[r][:, dt:dt + 1],
                                              inp[:, KD + dt, NCH - 1:NCH])

            # ---- MLP1 (shared weights) ----
            hids = [hid_p.tile([128, KH, NCH], F16, tag="hid")
                    for _ in range(B_loc)]
            for m in range(KH):
                if m in w1_tiles:
                    w1 = w1_tiles.pop(m)
                else:
                    w1 = w1_p.tile([128, 2 * KD, 128], F16, tag="w1")
                    nc.sync.dma_start(
                        w1[:],
                        p1_d[:, m * 128:(m + 1) * 128].rearrange(
                            "(kt p) m -> p kt m", p=128))
                for r in range(B_loc):
                    hp = h_ps.tile([128, NCH], F32, tag="hp")
                    for kt in range(2 * KD):
                        nc.tensor.matmul(hp[:], w1[:, kt, :], inps[r][:, kt, :],
                                         start=(kt == 0),
                                         stop=(kt == 2 * KD - 1))
                    nc.scalar.activation(hids[r][:, m, :], hp[:], AF.Relu,
                                         bias=b1_t[:, m:m + 1])

            # ---- MLP2 + router (shared weights) ----
            lps = [l_ps.tile([E, NCH], F32, tag="lp") for _ in range(B_loc)]
            for m2 in range(KD):
                w2 = w2_p.tile([128, KH, 128], F16, tag="w2")
                nc.sync.dma_start(
                    w2[:],
                    p2_d[:, m2 * 128:(m2 + 1) * 128].rearrange(
                        "(kt p) m -> p kt m", p=128))
                for r in range(B_loc):
                    xp = x_ps.tile([128, NCH], F32, tag="xp")
                    for kt in range(KH):
                        nc.tensor.matmul(xp[:], w2[:, kt, :], hids[r][:, kt, :],
                                         start=(kt == 0), stop=(kt == KH - 1))
                    xh32 = xh32_p.tile([128, NCH], F32, tag="xh32")
                    xh16 = xh16_p.tile([128, NCH], F16, tag="xh16")
                    nc.scalar.activation(xh32[:], xp[:], AF.Identity,
                                         bias=b2_t[:, m2:m2 + 1])
                    nc.vector.tensor_scalar_add(xh16[:], xp[:],
                                                b2_t[:, m2:m2 + 1])
                    nc.tensor.matmul(lps[r][:], wr_t[:, m2, :], xh16[:],
                                     start=(m2 == 0), stop=(m2 == KD - 1))
                    nc.sync.dma_start(
                        xhat_d[m2 * 128:(m2 + 1) * 128,
                               col0s[r]:col0s[r] + NCH],
                        xh32[:])

            for r in range(B_loc):
                lgs = lgs_p.tile([E, NCH], F32, tag="lgs")
                nc.scalar.activation(lgs[:], lps[r][:], AF.Identity,
                                     bias=wbpi_t[:], scale=pi_t[:])
                nc.sync.dma_start(lg_d[:, col0s[r]:col0s[r] + NCH], lgs[:])

    nc.compile()
    return nc


def prep_shared(pi, beta_raw, p1_w, p1_b, p2_w, p2_b, W_w, W_b):
    D = beta_raw.shape[0]
    H = p1_w.shape[0]
    E = W_w.shape[0]
    KD, KH = D // 128, H // 128
    beta = (1.0 / (1.0 + np.exp(-beta_raw.astype(np.float64)))).astype(np.float32)
    return {
        "p1wt": np.ascontiguousarray(p1_w.T).astype(np.float16),
        "p2wt": np.ascontiguousarray(p2_w.T).astype(np.float16),
        "wrt": np.ascontiguousarray(W_w.T).astype(np.float16),
        "beta": np.ascontiguousarray(beta.reshape(KD, 128).T),
        "b1": np.ascontiguousarray(
            np.asarray(p1_b, np.float32).reshape(KH, 128).T),
        "b2": np.ascontiguousarray(
            np.asarray(p2_b, np.float32).reshape(KD, 128).T),
        "wbpi": (np.asarray(W_b, np.float32)
                 * np.asarray(pi, np.float32)).reshape(E, 1),
        "pi": np.asarray(pi, np.float32).reshape(E, 1),
        "ident": np.eye(128, dtype=np.float16),
    }


def prep_core_inputs(seq_core, shared, T_pad=T_PAD):
    B_loc, T, D = seq_core.shape
    x = np.zeros((B_loc * T_pad, D), np.float16)
    for r in range(B_loc):
        x[r * T_pad:r * T_pad + T - 1] = seq_core[r, :T - 1].astype(np.float16)
    return {"x": x, **shared}


def postprocess(results, B, T, D, E, T_pad=T_PAD):
    n_cores = len(results)
    B_loc = B // n_cores
    logits = np.empty((T - 1, B, E), np.float32)
    x_hat = np.empty((T - 1, B, D), np.float32)
    for c, res in enumerate(results):
        xh = res["xhat"].reshape(D, B_loc, T_pad)
        lg = res["logits"].reshape(E, B_loc, T_pad)
        for r in range(B_loc):
            x_hat[:, c * B_loc + r, :] = xh[:, r, :T - 1].T
            logits[:, c * B_loc + r, :] = lg[:, r, :T - 1].T
    return logits, x_hat


_NC_CACHE = {}


def _get_nc(B_loc, T_pad, D, H, E, n_cores):
    key = (B_loc, T_pad, D, H, E, n_cores)
    if key not in _NC_CACHE:
        _NC_CACHE[key] = build(B_loc=B_loc, T_pad=T_pad, D=D, H=H, E=E,
                               n_cores=n_cores)
    return _NC_CACHE[key]


def kernel(seq, pi, beta_raw, p1_w, p1_b, p2_w, p2_b, W_w, W_b):
    seq = np.asarray(seq, np.float32)
    B, T, D = seq.shape
    H = np.asarray(p1_w).shape[0]
    E = np.asarray(W_w).shape[0]
    n_cores = N_CORES
    assert B % n_cores == 0 and B // n_cores == 2
    B_loc = B // n_cores

    nc = _get_nc(B_loc, T_PAD, D, H, E, n_cores)
    shared = prep_shared(np.asarray(pi), np.asarray(beta_raw),
                         np.asarray(p1_w), np.asarray(p1_b),
                         np.asarray(p2_w), np.asarray(p2_b),
                         np.asarray(W_w), np.asarray(W_b))
    in_maps = [prep_core_inputs(seq[c * B_loc:(c + 1) * B_loc], shared)
               for c in range(n_cores)]

    try:
        res = run_bass_kernel_spmd(nc, in_maps,
                                   core_ids=list(range(n_cores))).results
    except Exception:
        # transient device errors (e.g. NRT_EXEC_UNIT_UNRECOVERABLE) — retry
        res = run_bass_kernel_spmd(nc, in_maps,
                                   core_ids=list(range(n_cores))).results
    logits, x_hat = postprocess(res, B, T, D, E)
    return logits, x_hat
